# revision 28
# baseline (speedup 1.0000x reference)
"""Trainium2 Bass kernel for a pre-LN transformer block (B=128, T=256, D=384, H=6).

Sharding: data-parallel over batch across 8 NeuronCores (16 batches/core).

Design notes:
- Matmuls run in bf16 except FFN2 which runs fp8e4 DoubleRow (2x K per pass;
  measured faster than bf16 there, but NOT for FFN1/QKV where the 256-column
  no-FWL DoubleRow LDWEIGHTS cost exceeds the matmul savings). W1 carries a
  x16 scale so relu's fp8 output lands in e4m3 range, W2 carries another x16,
  and the 1/256 rescale is an ACT Identity op during FFN2 PSUM evacuation.
- Activations are produced feature-major (hT) via PE transposes so every matmul
  contracts over the partition dim with K=128 chunks.
- LN apply runs on the ACT engine as Identity(x*rstd + (-mu*rstd)) — per-
  partition scale/bias operands run at full ACT rate (tensor_scalar on
  DVE/GpSimd measured 6-15x slower).
- LN rsqrt = exp(-0.5*ln(var+eps)) and the softmax reciprocal = exp(-ln(d)),
  so every ACT function (ln/exp/relu/identity/copy) stays inside the
  natural_log_exp_and_others table set, pinned so the table-load pass never
  flip-flops sets (each load costs ~1.3us).
- Scores are computed TRANSPOSED (S^T[ts,tq] via lhsT=k, rhs=q) so exp writes
  attn^T directly and attn@v needs no PE transposes. Head pairs use K=64
  row-group packing (base partitions 0/64) so their score matmuls run
  concurrently on the PE. The causal mask is applied AFTER exp by zeroing
  ts>tq entries with GpSimd affine_selects (frees 12 PE matmuls/batch).
- Softmax denominators are per-head column sums of attn^T via ones-selector
  matmuls into a PSUM tile; 1/d is broadcast to head-pair partition ranges
  with a DRAM-bounce DMA. attn@v PSUM is evacuated RAW on the ACT engine
  (frees the PSUM ring without waiting on the DVE queue) and normalized
  SBUF->SBUF on DVE a step later.
- Elementwise work is spread across all four non-PE engines: LN stats/evacs
  and transposed-tile evacs on DVE, exp/relu(half)/LN-apply/oT-evac on ACT,
  causal mask + final residual adds on GpSimd, relu(other half) on DVE.
- 12-deep per-batch software pipeline: every SBUF tile feeding a PE matmul
  (LDWEIGHTS or rhs) is produced a full pipeline slot before the PE consumes
  it, and each engine's per-slot queue is ordered by consumption distance, so
  the PE rarely waits on the ACT/DVE FIFOs and HAM stays at full clock.
"""
import sys

for _p in ("/opt/trn_rl_repo",):
    if _p not in sys.path:
        sys.path.append(_p)

import numpy as np

import concourse.bacc as bacc
import concourse.bass as bass
import concourse.mybir as mybir
import concourse.tile as tile
from concourse.masks import make_identity

F32 = mybir.dt.float32
BF16 = mybir.dt.bfloat16
F8E4 = mybir.dt.float8e4
AF = mybir.ActivationFunctionType
ALU = mybir.AluOpType
DR = mybir.MatmulPerfMode.DoubleRow

N_CORES = 8
B, T, D, H, HD = 128, 256, 384, 6, 64
DF = 4 * D            # 1536
SB = B // N_CORES     # 16 batches per core
NEG = -1e9            # additive causal-mask value
EPS = 1e-5
FSCALE = 16.0         # fp8 scale on fT (relu out) and W2; evac rescale 1/256
PIN_SET = "natural_log_exp_and_others"

_orig_gat = bacc.get_activation_tables


def _pinned_gat(arch):
    tabs = _orig_gat(arch)
    fns = tabs.get(PIN_SET) or set()
    if AF.Exp in fns and AF.Ln in fns and AF.Relu in fns and AF.Identity in fns:
        tabs = {k: (v if k == PIN_SET else set()) for k, v in tabs.items()}
    return tabs


bacc.get_activation_tables = _pinned_gat

# pipeline stage offsets within a slot: slot s runs stage k for batch s-OFF[k]
N_STAGES = 12  # x, ln1, tr1, qkv, sc, rs, bc, av, pr, tr2, f1, f2


def build_program(reps: int = 1, use_bqkv=False, use_bp=False, use_b1=False, use_b2=False):
    nc = bacc.Bacc("TRN2", target_bir_lowering=False, debug=False)

    x_d = nc.dram_tensor("x", [SB, T, D], F32, kind="ExternalInput").ap()
    wqkv_d = nc.dram_tensor("wqkv", [3, 128, 3 * D], BF16, kind="ExternalInput").ap()
    wp_d = nc.dram_tensor("wp", [3, 128, D], BF16, kind="ExternalInput").ap()
    w1_d = nc.dram_tensor("w1", [3, 128, DF], BF16, kind="ExternalInput").ap()
    w2_d = nc.dram_tensor("w2", [12, 128, D], F8E4, kind="ExternalInput").ap()
    bias_d = {}
    for name, use, n in (("bqkv", use_bqkv, 3 * D), ("bp", use_bp, D),
                         ("b1", use_b1, DF), ("b2", use_b2, D)):
        if use:
            bias_d[name] = nc.dram_tensor(name, [1, n], BF16, kind="ExternalInput").ap()
    rs_scr = nc.dram_tensor("rs_scr", [SB, 6, 256], F32).ap()  # internal scratch
    out_d = nc.dram_tensor("out", [SB, T, D], F32, kind="ExternalOutput").ap()

    with tile.TileContext(nc) as tc:
        _emit(nc, tc, x_d, wqkv_d, wp_d, w1_d, w2_d, bias_d, rs_scr, out_d, reps)
    nc.compile()
    return nc


def _emit(nc, tc, x_d, wqkv_d, wp_d, w1_d, w2_d, bias_d, rs_scr, out_d, reps):
    from contextlib import ExitStack
    ctx = ExitStack()
    with ctx:
        wpool = ctx.enter_context(tc.tile_pool(name="w", bufs=1))
        sb = ctx.enter_context(tc.tile_pool(name="sb", bufs=3))
        sbx = ctx.enter_context(tc.tile_pool(name="sbx", bufs=9))
        stats = ctx.enter_context(tc.tile_pool(name="stats", bufs=4))
        ps_mm = ctx.enter_context(tc.tile_pool(name="ps_mm", bufs=2, space="PSUM"))
        ps_sc = ctx.enter_context(tc.tile_pool(name="ps_sc", bufs=3, space="PSUM"))
        ps_tr = ctx.enter_context(tc.tile_pool(name="ps_tr", bufs=1, space="PSUM"))
        ps_ot = ctx.enter_context(tc.tile_pool(name="ps_ot", bufs=2, space="PSUM"))

        # --- constants ---
        for cval in (0.0, EPS):
            cap = wpool.tile([128, 1], F32, tag=f"const{cval}")
            nc.vector.memset(cap, cval)
            nc.const_aps.aps[(F32, cval)] = cap
        ident = wpool.tile([128, 128], BF16, tag="ident")
        make_identity(nc, ident)
        # transposed causal mask for S^T[ts, tq]: 0 where ts <= tq, NEG below diag
        trimaskT = wpool.tile([128, 128], BF16, tag="trimaskT")
        nc.gpsimd.memset(trimaskT, NEG)
        nc.gpsimd.affine_select(
            out=trimaskT, in_=trimaskT, compare_op=ALU.is_gt, fill=0.0,
            base=0, pattern=[[-1, 128]], channel_multiplier=1,
        )
        # per-head ones-selector columns for PSUM-row sums: sel6[:, h, j] = (j == h)
        sel6 = wpool.tile([128, 6, 6], BF16, tag="sel6")
        nc.gpsimd.memset(sel6, 0.0)
        for h in range(6):
            nc.gpsimd.memset(sel6[:, h, h : h + 1], 1.0)

        # --- weights ---
        wqkv_sb = wpool.tile([128, 3, 3 * D], BF16, tag="wqkv")
        wp_sb = wpool.tile([128, 3, D], BF16, tag="wp")
        w1_sb = wpool.tile([128, 3, DF], BF16, tag="w1")
        w2_sb = wpool.tile([128, 12, D], F8E4, tag="w2")
        for c in range(3):
            nc.sync.dma_start(out=wqkv_sb[:, c, :], in_=wqkv_d[c])
            nc.sync.dma_start(out=wp_sb[:, c, :], in_=wp_d[c])
            nc.sync.dma_start(out=w1_sb[:, c, :], in_=w1_d[c])
        for c in range(12):
            nc.sync.dma_start(out=w2_sb[:, c, :], in_=w2_d[c])
        bias_sb = {}
        ones = None
        if bias_d:
            ones = wpool.tile([1, T], BF16, tag="ones")
            nc.vector.memset(ones, 1.0)
            for name, ap in bias_d.items():
                t = wpool.tile([1, ap.shape[1]], BF16, tag=f"b_{name}")
                nc.sync.dma_start(out=t, in_=ap)
                bias_sb[name] = t

        st = {}

        # ---------- stage bodies ----------
        def s_x(b):
            x_t = []
            for tt in range(2):
                xt = sbx.tile([128, D], F32, tag=f"x{tt}")
                nc.sync.dma_start(out=xt, in_=x_d[b, 128 * tt : 128 * (tt + 1), :])
                x_t.append(xt)
            st[("x", b)] = x_t

        def ln_pre(x_tiles, key):
            mv = stats.tile([128, 2, 2], F32, tag=f"mv{key}")
            for tt in range(2):
                bst = stats.tile([128, 6], F32, tag=f"st{key}")
                nc.vector.bn_stats(out=bst, in_=x_tiles[tt])
                nc.vector.bn_aggr(out=mv[:, tt, :], in_=bst)
            lnv = stats.tile([128, 2], F32, tag=f"lnv{key}")
            nc.scalar.activation(out=lnv, in_=mv[:, :, 1], func=AF.Ln, bias=EPS)
            rstd = stats.tile([128, 2], F32, tag=f"rstd{key}")
            nc.scalar.activation(out=rstd, in_=lnv, func=AF.Exp, scale=-0.5)
            negmu = stats.tile([128, 2], F32, tag=f"negmu{key}")
            nc.scalar.activation(out=negmu, in_=mv[:, :, 0], func=AF.Copy, scale=-1.0)
            nmr = stats.tile([128, 2], F32, tag=f"nmr{key}")
            nc.vector.tensor_mul(out=nmr, in0=negmu, in1=rstd)
            h_t = []
            for tt in range(2):
                h = sb.tile([128, D], BF16, tag=f"h{key}{tt}")
                nc.scalar.activation(
                    out=h, in_=x_tiles[tt], func=AF.Identity,
                    scale=rstd[:, tt : tt + 1], bias=nmr[:, tt : tt + 1],
                )
                h_t.append(h)
            return h_t

        def s_ln1(b):
            st[("h1", b)] = ln_pre(st[("x", b)], "a")

        def ln_tr(h_t, key, dtype=BF16):
            """h tiles (token-major bf16) -> hT [128, 3, T] (feature-major)."""
            trp = ps_tr.tile([128, 2, 3, 128], BF16, tag="tr")
            for tt in range(2):
                for c in range(3):
                    nc.tensor.transpose(trp[:, tt, c, :],
                                        h_t[tt][:, 128 * c : 128 * (c + 1)], ident)
            hT = sb.tile([128, 3, T], dtype, tag=f"{key}T", bufs=4)
            # single ACT evac: out AP ordered (tt, c, col) to match trp layout
            out_ap = bass.AP(
                tensor=hT.tensor, offset=hT.offset,
                ap=[list(hT.ap[0]), [128, 2], [T, 3], [1, 128]],
            )
            nc.vector.tensor_copy(out=out_ap, in_=trp)
            return hT

        def s_tr1(b):
            st[("hT", b)] = ln_tr(st.pop(("h1", b)), "h")

        def s_qkv(b):
            hT = st[("hT", b)]
            qkT = sb.tile([128, 6, T], BF16, tag="qkT", bufs=4)  # q: m 0..2, k: m 3..5
            nq = 4 if "bqkv" in bias_sb else 3
            for mp in range(3):  # pairs of m-chunks share one PSUM bank
                ps = ps_mm.tile([128, 512], F32, tag="mm")
                for sub in range(2):
                    m = 2 * mp + sub
                    reg = ps[:, 256 * sub : 256 * (sub + 1)]
                    for c in range(3):
                        nc.tensor.matmul(reg, lhsT=wqkv_sb[:, c, 128 * m : 128 * (m + 1)],
                                         rhs=hT[:, c, :], start=(c == 0), stop=(c == nq - 1))
                    if "bqkv" in bias_sb:
                        nc.tensor.matmul(reg, lhsT=bias_sb["bqkv"][:, 128 * m : 128 * (m + 1)],
                                         rhs=ones[:, :T], start=False, stop=True)
                nc.vector.tensor_copy(out=qkT[:, 2 * mp : 2 * mp + 2, :], in_=ps)
            v_sb = []
            for tt in range(2):
                ps = ps_mm.tile([128, D], F32, tag="mm")
                for c in range(3):
                    nc.tensor.matmul(ps, lhsT=hT[:, c, 128 * tt : 128 * (tt + 1)],
                                     rhs=wqkv_sb[:, c, 2 * D : 3 * D],
                                     start=(c == 0), stop=(c == nq - 1))
                if "bqkv" in bias_sb:
                    nc.tensor.matmul(ps, lhsT=ones[:, :128],
                                     rhs=bias_sb["bqkv"][:, 2 * D : 3 * D],
                                     start=False, stop=True)
                vt = sbx.tile([128, D], BF16, tag=f"v{tt}", bufs=8)
                nc.vector.tensor_copy(out=vt, in_=ps)
                v_sb.append(vt)
            st.pop(("hT", b))
            st[("qk", b)] = qkT
            st[("v", b)] = v_sb

        def s_sc(b):
            """S^T layout per head: cols 0:256 = (ts c0) x (tq 0:256);
            cols 256:384 = (ts c1) x (tq c1). Head pairs run CONCURRENTLY on
            the PE (K=64 each, auto row-group tiling from base_partition 0/64).
            Causal mask applied post-exp by zeroing ts>tq on GpSimd."""
            qkT = st.pop(("qk", b))
            attnT = sb.tile([128, 6, 384], BF16, tag="attnT", bufs=6)
            for m in range(3):
                pair = []
                for sub in range(2):
                    h = 2 * m + sub
                    po = sub * 64
                    qh = qkT[po : po + 64, m, :]
                    kh = qkT[po : po + 64, 3 + m, :]
                    S = ps_sc.tile([128, 384], F32, tag="sc")
                    pair.append((h, S, qh, kh))
                for h, S, qh, kh in pair:
                    nc.tensor.matmul(S[:, 0:256], lhsT=kh[:, 0:128], rhs=qh,
                                     start=True, stop=False)
                for h, S, qh, kh in pair:
                    nc.tensor.matmul(S[:, 256:384], lhsT=kh[:, 128:256],
                                     rhs=qh[:, 128:256], start=False, stop=True)
                for h, S, qh, kh in pair:
                    nc.scalar.activation(out=attnT[:, h, :], in_=S, func=AF.Exp)
                    # zero masked (ts > tq) entries of blocks 0:128 and 256:384
                    for blk in (attnT[:, h, 0:128], attnT[:, h, 256:384]):
                        nc.gpsimd.affine_select(
                            out=blk, in_=blk, compare_op=ALU.is_ge, fill=0.0,
                            base=0, pattern=[[1, 128]], channel_multiplier=-1,
                        )
            st[("at", b)] = attnT

        def s_rs(b):
            """rowsum reciprocals -> DRAM bounce broadcast to head-pair rows."""
            attnT = st[("at", b)]
            rs_ps = ps_sc.tile([6, 256], F32, tag="sc")
            for h in range(H):
                nc.tensor.matmul(rs_ps, lhsT=sel6[:, h, :], rhs=attnT[:, h, 0:256],
                                 start=(h == 0), stop=False, skip_group_check=True)
                nc.tensor.matmul(rs_ps[:, 128:256], lhsT=sel6[:, h, :],
                                 rhs=attnT[:, h, 256:384],
                                 start=False, stop=(h == H - 1), skip_group_check=True)
            # 1/d via exp(-ln d) on ACT: keeps the evac off the DVE queue and
            # both funcs stay in the pinned table set
            lnd = stats.tile([6, 256], F32, tag="lnd")
            nc.scalar.activation(out=lnd, in_=rs_ps, func=AF.Ln)
            rsi = stats.tile([6, 256], F32, tag="rsi")
            nc.scalar.activation(out=rsi, in_=lnd, func=AF.Exp, scale=-1.0)
            nc.sync.dma_start(out=rs_scr[b], in_=rsi)

        def s_bc(b):
            bcs = []
            for m in range(3):
                src_ap = rs_scr[b, 2 * m : 2 * m + 2, :]
                src = bass.AP(tensor=src_ap.tensor, offset=src_ap.offset,
                              ap=[list(src_ap.ap[0]), [0, 64], list(src_ap.ap[1])])
                bc_m = sb.tile([128, 256], F32, tag=f"rsbc{m}", bufs=3)
                nc.sync.dma_start(out=bc_m, in_=src)
                bcs.append(bc_m)
            st[("bc", b)] = bcs

        def s_av(b):
            attnT = st.pop(("at", b))
            v_sb = st.pop(("v", b))
            bcs = st.pop(("bc", b))
            oTs = sb.tile([128, 3, T], BF16, tag="oTs", bufs=4)
            oTr = sb.tile([128, 3, T], F32, tag="oTr", bufs=2)
            for m in range(3):
                oT_ps = ps_ot.tile([128, T], F32, tag="ot")
                for sub in range(2):
                    h = 2 * m + sub
                    po = sub * 64
                    tp = (0, po)
                    nc.tensor.matmul(oT_ps[po : po + 64, :],
                                     lhsT=v_sb[0][:, HD * h : HD * (h + 1)],
                                     rhs=attnT[:, h, 0:256],
                                     start=True, stop=False, tile_position=tp)
                    nc.tensor.matmul(oT_ps[po : po + 64, 128:256],
                                     lhsT=v_sb[1][:, HD * h : HD * (h + 1)],
                                     rhs=attnT[:, h, 256:384],
                                     start=False, stop=True, tile_position=tp)
                # evac on ACT frees the ot bank without waiting on the DVE queue
                nc.scalar.copy(out=oTr[:, m, :], in_=oT_ps)
            for m in range(3):
                nc.vector.tensor_mul(out=oTs[:, m, :], in0=oTr[:, m, :], in1=bcs[m])
            st[("o", b)] = oTs

        def s_pr(b):
            oTs = st.pop(("o", b))
            x_t = st.pop(("x", b))
            npj = 4 if "bp" in bias_sb else 3
            x2_t = []
            for tt in range(2):
                ps = ps_mm.tile([128, D], F32, tag="mm")
                for c in range(3):
                    nc.tensor.matmul(ps, lhsT=oTs[:, c, 128 * tt : 128 * (tt + 1)],
                                     rhs=wp_sb[:, c, :], start=(c == 0), stop=(c == npj - 1))
                if "bp" in bias_sb:
                    nc.tensor.matmul(ps, lhsT=ones[:, :128], rhs=bias_sb["bp"],
                                     start=False, stop=True)
                x2 = sbx.tile([128, D], F32, tag=f"x2_{tt}", bufs=6)
                nc.vector.tensor_add(out=x2, in0=x_t[tt], in1=ps)
                x2_t.append(x2)
            st[("h2", b)] = ln_pre(x2_t, "b")
            st[("x2", b)] = x2_t

        def s_tr2(b):
            st[("gT", b)] = ln_tr(st.pop(("h2", b)), "g")

        def s_f1(b):
            h2T = st.pop(("gT", b))
            fT = sb.tile([128, 12, T], F8E4, tag="fT", bufs=4)
            nf = 4 if "b1" in bias_sb else 3
            for mp in range(6):
                ps = ps_mm.tile([128, 512], F32, tag="mm")
                for sub in range(2):
                    m = 2 * mp + sub
                    reg = ps[:, 256 * sub : 256 * (sub + 1)]
                    for c in range(3):
                        nc.tensor.matmul(reg, lhsT=w1_sb[:, c, 128 * m : 128 * (m + 1)],
                                         rhs=h2T[:, c, :], start=(c == 0), stop=(c == nf - 1))
                    if "b1" in bias_sb:
                        nc.tensor.matmul(reg, lhsT=bias_sb["b1"][:, 128 * m : 128 * (m + 1)],
                                         rhs=ones[:, :T], start=False, stop=True)
                if mp % 2 == 0:
                    nc.scalar.activation(out=fT[:, 2 * mp : 2 * mp + 2, :], in_=ps,
                                         func=AF.Relu)
                else:
                    nc.vector.tensor_scalar_max(out=fT[:, 2 * mp : 2 * mp + 2, :],
                                                in0=ps, scalar1=0.0)
            st[("f", b)] = fT

        def s_f2(b):
            fT = st.pop(("f", b))
            x2_t = st.pop(("x2", b))
            for tt in range(2):
                ps = ps_mm.tile([128, D], F32, tag="mm")
                for kp in range(6):
                    kc = 2 * kp
                    nc.tensor.matmul(
                        ps,
                        lhsT=fT[:, kc : kc + 2, 128 * tt : 128 * (tt + 1)],
                        rhs=w2_sb[:, kc : kc + 2, :],
                        start=(kp == 0), stop=(kp == 5), perf_mode=DR,
                    )
                if "b2" in bias_sb:
                    # b2 pre-scaled by 256 host-side; K=1 bf16 matmul into group
                    nc.tensor.matmul(ps, lhsT=ones[:1, :128], rhs=bias_sb["b2"],
                                     start=False, stop=True, skip_group_check=True)
                ffs = sb.tile([128, D], F32, tag=f"ffs{tt}", bufs=2)
                nc.scalar.activation(out=ffs, in_=ps, func=AF.Identity,
                                     scale=1.0 / (FSCALE * FSCALE))
                fo = sb.tile([128, D], F32, tag=f"fo{tt}", bufs=2)
                nc.gpsimd.tensor_add(out=fo, in0=x2_t[tt], in1=ffs)
                nc.sync.dma_start(out=out_d[b, 128 * tt : 128 * (tt + 1), :], in_=fo)

        STAGES = [s_x, s_ln1, s_tr1, s_qkv, s_sc, s_rs, s_bc, s_av, s_pr, s_tr2, s_f1, s_f2]

        def emit_all():
            for s in range(SB + N_STAGES - 1):
                for k, fn in enumerate(STAGES):
                    b = s - k
                    if 0 <= b < SB:
                        fn(b)

        if reps == 1:
            emit_all()
        else:
            with tc.For_i(0, reps) as _:
                emit_all()


def prep_weights(Wq, Wk, Wv, Wp, bp, W1, b1, W2, b2, g1, be1, g2, be2):
    """Host-side weight folding. Returns dict of device arrays + bias flags."""
    import ml_dtypes
    bf = ml_dtypes.bfloat16
    f8 = ml_dtypes.float8_e4m3
    Wq = np.asarray(Wq, np.float32)
    Wk = np.asarray(Wk, np.float32)
    Wv = np.asarray(Wv, np.float32)
    Wp = np.asarray(Wp, np.float32)
    W1 = np.asarray(W1, np.float32)
    W2 = np.asarray(W2, np.float32)
    g1 = np.asarray(g1, np.float32); be1 = np.asarray(be1, np.float32)
    g2 = np.asarray(g2, np.float32); be2 = np.asarray(be2, np.float32)
    bp = np.asarray(bp, np.float32); b1 = np.asarray(b1, np.float32)
    b2 = np.asarray(b2, np.float32)

    # [H, D, HD] -> [D, H*HD]
    Wq2 = Wq.transpose(1, 0, 2).reshape(D, D)
    Wk2 = Wk.transpose(1, 0, 2).reshape(D, D)
    Wv2 = Wv.transpose(1, 0, 2).reshape(D, D)
    Wqkv = np.concatenate([Wq2, Wk2, Wv2], axis=1)          # [D, 3D]
    bqkv = be1 @ Wqkv                                       # bias from LN1 beta
    Wqkv = g1[:, None] * Wqkv                               # fold LN1 gamma
    scale = 1.0 / np.sqrt(np.float32(D))
    Wqkv[:, :D] *= scale                                    # fold score scale into q
    bqkv = bqkv.copy()
    bqkv[:D] *= scale

    W1e = g2[:, None] * W1 * FSCALE                         # fold LN2 gamma + fp8 scale
    b1e = (b1 + be2 @ W1) * FSCALE                          # fold LN2 beta

    W2s = np.clip(W2 * FSCALE, -240.0, 240.0)               # fp8 range

    out = {
        "wqkv": np.ascontiguousarray(Wqkv.reshape(3, 128, 3 * D)).astype(bf),
        "wp": np.ascontiguousarray(Wp.reshape(3, 128, D)).astype(bf),
        "w1": np.ascontiguousarray(W1e.reshape(3, 128, DF)).astype(bf),
        "w2": np.ascontiguousarray(W2s.reshape(12, 128, D)).astype(f8),
    }
    flags = {}
    b2s = b2 * (FSCALE * FSCALE)  # survives the 1/256 evac rescale
    for name, arr in (("bqkv", bqkv), ("bp", bp), ("b1", b1e), ("b2", b2s)):
        if np.any(arr != 0):
            out[name] = arr.reshape(1, -1).astype(bf)
            flags[f"use_{name}"] = True
        else:
            flags[f"use_{name}"] = False
    return out, flags


_CACHE = {}


def get_program(flags, reps=1):
    key = (reps, tuple(sorted(flags.items())))
    if key not in _CACHE:
        _CACHE[key] = build_program(reps=reps, **flags)
    return _CACHE[key]


def make_in_maps(x, w):
    in_maps = []
    for c in range(N_CORES):
        m = {"x": np.ascontiguousarray(np.asarray(x, np.float32)[c * SB : (c + 1) * SB])}
        m.update(w)
        in_maps.append(m)
    return in_maps


def kernel(x, Wq, Wk, Wv, Wp, bp, W1, b1, W2, b2, g1, be1, g2, be2):
    from concourse.bass_utils import run_bass_kernel_spmd

    w, flags = prep_weights(Wq, Wk, Wv, Wp, bp, W1, b1, W2, b2, g1, be1, g2, be2)
    nc = get_program(flags, reps=1)
    in_maps = make_in_maps(x, w)
    res = run_bass_kernel_spmd(nc, in_maps, list(range(N_CORES)))
    return np.concatenate([res.results[c]["out"] for c in range(N_CORES)], axis=0)


# revision 30
# speedup vs baseline: 1.0694x; 1.0694x over previous
"""Trainium2 Bass kernel for a pre-LN transformer block (B=128, T=256, D=384, H=6).

Sharding: data-parallel over batch across 8 NeuronCores (16 batches/core).

Design notes:
- Matmuls run in bf16 except FFN2 which runs fp8e4 DoubleRow (2x K per pass;
  measured faster than bf16 there, but NOT for FFN1/QKV where the 256-column
  no-FWL DoubleRow LDWEIGHTS cost exceeds the matmul savings). W1 carries a
  x16 scale so relu's fp8 output lands in e4m3 range, W2 carries another x16,
  and the 1/256 rescale is an ACT Identity op during FFN2 PSUM evacuation.
- Activations are produced feature-major (hT) via PE transposes so every matmul
  contracts over the partition dim with K=128 chunks.
- LN apply runs on the ACT engine as Identity(x*rstd + (-mu*rstd)) — per-
  partition scale/bias operands run at full ACT rate (tensor_scalar on
  DVE/GpSimd measured 6-15x slower).
- LN rsqrt = exp(-0.5*ln(var+eps)) and the softmax reciprocal = exp(-ln(d)),
  so every ACT function (ln/exp/relu/identity/copy) stays inside the
  natural_log_exp_and_others table set, pinned so the table-load pass never
  flip-flops sets (each load costs ~1.3us).
- Scores are computed TRANSPOSED (S^T[ts,tq] via lhsT=k, rhs=q) so exp writes
  attn^T directly and attn@v needs no PE transposes. Head pairs use K=64
  row-group packing (base partitions 0/64) so their score matmuls run
  concurrently on the PE. The causal mask is applied AFTER exp by zeroing
  ts>tq entries with GpSimd affine_selects (frees 12 PE matmuls/batch).
- Softmax denominators are per-head column sums of attn^T via ones-selector
  matmuls into a PSUM tile; 1/d is broadcast to head-pair partition ranges
  with a DRAM-bounce DMA. attn@v PSUM is evacuated RAW on the ACT engine
  (frees the PSUM ring without waiting on the DVE queue) and normalized
  SBUF->SBUF on DVE a step later.
- Elementwise work is spread across all four non-PE engines: LN stats/evacs
  and transposed-tile evacs on DVE, exp/relu(half)/LN-apply/oT-evac on ACT,
  causal mask + final residual adds on GpSimd, relu(other half) on DVE.
- 12-deep per-batch software pipeline: every SBUF tile feeding a PE matmul
  (LDWEIGHTS or rhs) is produced a full pipeline slot before the PE consumes
  it, and each engine's per-slot queue is ordered by consumption distance, so
  the PE rarely waits on the ACT/DVE FIFOs and HAM stays at full clock.
"""
import sys

for _p in ("/opt/trn_rl_repo",):
    if _p not in sys.path:
        sys.path.append(_p)

import numpy as np

import concourse.bacc as bacc
import concourse.bass as bass
import concourse.mybir as mybir
import concourse.tile as tile
from concourse.masks import make_identity

F32 = mybir.dt.float32
BF16 = mybir.dt.bfloat16
F8E4 = mybir.dt.float8e4
AF = mybir.ActivationFunctionType
ALU = mybir.AluOpType
DR = mybir.MatmulPerfMode.DoubleRow

N_CORES = 8
B, T, D, H, HD = 128, 256, 384, 6, 64
DF = 4 * D            # 1536
SB = B // N_CORES     # 16 batches per core
NEG = -1e9            # additive causal-mask value
EPS = 1e-5
FSCALE = 16.0         # fp8 scale on fT (relu out) and W2; evac rescale 1/256
PIN_SET = "natural_log_exp_and_others"

_orig_gat = bacc.get_activation_tables


def _pinned_gat(arch):
    tabs = _orig_gat(arch)
    fns = tabs.get(PIN_SET) or set()
    if AF.Exp in fns and AF.Ln in fns and AF.Relu in fns and AF.Identity in fns:
        tabs = {k: (v if k == PIN_SET else set()) for k, v in tabs.items()}
    return tabs


bacc.get_activation_tables = _pinned_gat

# pipeline stage offsets within a slot: slot s runs stage k for batch s-OFF[k]
N_STAGES = 12  # x, ln1, tr1, qkv, sc, rs, bc, av, pr, tr2, f1, f2


def build_program(reps: int = 1, use_bqkv=False, use_bp=False, use_b1=False, use_b2=False):
    nc = bacc.Bacc("TRN2", target_bir_lowering=False, debug=False)

    x_d = nc.dram_tensor("x", [SB, T, D], F32, kind="ExternalInput").ap()
    wqkv_d = nc.dram_tensor("wqkv", [3, 128, 3 * D], BF16, kind="ExternalInput").ap()
    wp_d = nc.dram_tensor("wp", [3, 128, D], BF16, kind="ExternalInput").ap()
    w1_d = nc.dram_tensor("w1", [3, 128, DF], BF16, kind="ExternalInput").ap()
    w2_d = nc.dram_tensor("w2", [12, 128, D], F8E4, kind="ExternalInput").ap()
    bias_d = {}
    for name, use, n in (("bqkv", use_bqkv, 3 * D), ("bp", use_bp, D),
                         ("b1", use_b1, DF), ("b2", use_b2, D)):
        if use:
            bias_d[name] = nc.dram_tensor(name, [1, n], BF16, kind="ExternalInput").ap()
    rs_scr = nc.dram_tensor("rs_scr", [SB, 6, 256], F32).ap()  # internal scratch
    out_d = nc.dram_tensor("out", [SB, T, D], F32, kind="ExternalOutput").ap()

    with tile.TileContext(nc) as tc:
        _emit(nc, tc, x_d, wqkv_d, wp_d, w1_d, w2_d, bias_d, rs_scr, out_d, reps)
    nc.compile()
    return nc


def _emit(nc, tc, x_d, wqkv_d, wp_d, w1_d, w2_d, bias_d, rs_scr, out_d, reps):
    from contextlib import ExitStack
    ctx = ExitStack()
    with ctx:
        wpool = ctx.enter_context(tc.tile_pool(name="w", bufs=1))
        sb = ctx.enter_context(tc.tile_pool(name="sb", bufs=3))
        sbx = ctx.enter_context(tc.tile_pool(name="sbx", bufs=9))
        stats = ctx.enter_context(tc.tile_pool(name="stats", bufs=4))
        ps_mm = ctx.enter_context(tc.tile_pool(name="ps_mm", bufs=2, space="PSUM"))
        ps_sc = ctx.enter_context(tc.tile_pool(name="ps_sc", bufs=3, space="PSUM"))
        ps_tr = ctx.enter_context(tc.tile_pool(name="ps_tr", bufs=1, space="PSUM"))
        ps_ot = ctx.enter_context(tc.tile_pool(name="ps_ot", bufs=2, space="PSUM"))

        # --- constants ---
        for cval in (0.0, EPS):
            cap = wpool.tile([128, 1], F32, tag=f"const{cval}")
            nc.vector.memset(cap, cval)
            nc.const_aps.aps[(F32, cval)] = cap
        ident = wpool.tile([128, 128], BF16, tag="ident")
        make_identity(nc, ident)
        # transposed causal mask for S^T[ts, tq]: 0 where ts <= tq, NEG below diag
        trimaskT = wpool.tile([128, 128], BF16, tag="trimaskT")
        nc.gpsimd.memset(trimaskT, NEG)
        nc.gpsimd.affine_select(
            out=trimaskT, in_=trimaskT, compare_op=ALU.is_gt, fill=0.0,
            base=0, pattern=[[-1, 128]], channel_multiplier=1,
        )
        # per-head ones-selector columns for PSUM-row sums: sel6[:, h, j] = (j == h)
        sel6 = wpool.tile([128, 6, 6], BF16, tag="sel6")
        nc.gpsimd.memset(sel6, 0.0)
        for h in range(6):
            nc.gpsimd.memset(sel6[:, h, h : h + 1], 1.0)

        # --- weights ---
        wqkv_sb = wpool.tile([128, 3, 3 * D], BF16, tag="wqkv")
        wp_sb = wpool.tile([128, 3, D], BF16, tag="wp")
        w1_sb = wpool.tile([128, 3, DF], BF16, tag="w1")
        w2_sb = wpool.tile([128, 12, D], F8E4, tag="w2")
        for c in range(3):
            nc.sync.dma_start(out=wqkv_sb[:, c, :], in_=wqkv_d[c])
            nc.sync.dma_start(out=wp_sb[:, c, :], in_=wp_d[c])
            nc.sync.dma_start(out=w1_sb[:, c, :], in_=w1_d[c])
        for c in range(12):
            nc.sync.dma_start(out=w2_sb[:, c, :], in_=w2_d[c])
        bias_sb = {}
        ones = None
        if bias_d:
            ones = wpool.tile([1, T], BF16, tag="ones")
            nc.vector.memset(ones, 1.0)
            for name, ap in bias_d.items():
                t = wpool.tile([1, ap.shape[1]], BF16, tag=f"b_{name}")
                nc.sync.dma_start(out=t, in_=ap)
                bias_sb[name] = t

        st = {}

        # ---------- stage bodies ----------
        def s_x(b):
            x_t = []
            for tt in range(2):
                xt = sbx.tile([128, D], F32, tag=f"x{tt}")
                nc.sync.dma_start(out=xt, in_=x_d[b, 128 * tt : 128 * (tt + 1), :])
                x_t.append(xt)
            st[("x", b)] = x_t

        def ln_pre(x_tiles, key):
            mv = stats.tile([128, 2, 2], F32, tag=f"mv{key}")
            for tt in range(2):
                bst = stats.tile([128, 6], F32, tag=f"st{key}")
                nc.vector.bn_stats(out=bst, in_=x_tiles[tt])
                nc.vector.bn_aggr(out=mv[:, tt, :], in_=bst)
            lnv = stats.tile([128, 2], F32, tag=f"lnv{key}")
            nc.scalar.activation(out=lnv, in_=mv[:, :, 1], func=AF.Ln, bias=EPS)
            rstd = stats.tile([128, 2], F32, tag=f"rstd{key}")
            nc.scalar.activation(out=rstd, in_=lnv, func=AF.Exp, scale=-0.5)
            negmu = stats.tile([128, 2], F32, tag=f"negmu{key}")
            nc.scalar.activation(out=negmu, in_=mv[:, :, 0], func=AF.Copy, scale=-1.0)
            nmr = stats.tile([128, 2], F32, tag=f"nmr{key}")
            nc.vector.tensor_mul(out=nmr, in0=negmu, in1=rstd)
            h_t = []
            for tt in range(2):
                h = sb.tile([128, D], BF16, tag=f"h{key}{tt}")
                nc.scalar.activation(
                    out=h, in_=x_tiles[tt], func=AF.Identity,
                    scale=rstd[:, tt : tt + 1], bias=nmr[:, tt : tt + 1],
                )
                h_t.append(h)
            return h_t

        def s_ln1(b):
            st[("h1", b)] = ln_pre(st[("x", b)], "a")

        def ln_tr(h_t, key, dtype=BF16):
            """h tiles (token-major bf16) -> hT [128, 3, T] (feature-major)."""
            trp = ps_tr.tile([128, 2, 3, 128], BF16, tag="tr")
            for tt in range(2):
                for c in range(3):
                    nc.tensor.transpose(trp[:, tt, c, :],
                                        h_t[tt][:, 128 * c : 128 * (c + 1)], ident)
            hT = sb.tile([128, 3, T], dtype, tag=f"{key}T", bufs=4)
            # single ACT evac: out AP ordered (tt, c, col) to match trp layout
            out_ap = bass.AP(
                tensor=hT.tensor, offset=hT.offset,
                ap=[list(hT.ap[0]), [128, 2], [T, 3], [1, 128]],
            )
            nc.vector.tensor_copy(out=out_ap, in_=trp)
            return hT

        def s_tr1(b):
            st[("hT", b)] = ln_tr(st.pop(("h1", b)), "h")

        def s_qkv(b):
            hT = st[("hT", b)]
            qkT = sb.tile([128, 6, T], BF16, tag="qkT", bufs=4)  # q: m 0..2, k: m 3..5
            nq = 4 if "bqkv" in bias_sb else 3
            for mp in range(3):  # pairs of m-chunks share one PSUM bank
                ps = ps_mm.tile([128, 512], F32, tag="mm")
                for sub in range(2):
                    m = 2 * mp + sub
                    reg = ps[:, 256 * sub : 256 * (sub + 1)]
                    for c in range(3):
                        nc.tensor.matmul(reg, lhsT=wqkv_sb[:, c, 128 * m : 128 * (m + 1)],
                                         rhs=hT[:, c, :], start=(c == 0), stop=(c == nq - 1))
                    if "bqkv" in bias_sb:
                        nc.tensor.matmul(reg, lhsT=bias_sb["bqkv"][:, 128 * m : 128 * (m + 1)],
                                         rhs=ones[:, :T], start=False, stop=True)
                nc.vector.tensor_copy(out=qkT[:, 2 * mp : 2 * mp + 2, :], in_=ps)
            v_sb = []
            for tt in range(2):
                ps = ps_mm.tile([128, D], F32, tag="mm")
                for c in range(3):
                    nc.tensor.matmul(ps, lhsT=hT[:, c, 128 * tt : 128 * (tt + 1)],
                                     rhs=wqkv_sb[:, c, 2 * D : 3 * D],
                                     start=(c == 0), stop=(c == nq - 1))
                if "bqkv" in bias_sb:
                    nc.tensor.matmul(ps, lhsT=ones[:, :128],
                                     rhs=bias_sb["bqkv"][:, 2 * D : 3 * D],
                                     start=False, stop=True)
                vt = sbx.tile([128, D], BF16, tag=f"v{tt}", bufs=8)
                nc.vector.tensor_copy(out=vt, in_=ps)
                v_sb.append(vt)
            st.pop(("hT", b))
            st[("qk", b)] = qkT
            st[("v", b)] = v_sb

        def s_sc(b):
            """S^T layout per head: cols 0:256 = (ts c0) x (tq 0:256);
            cols 256:384 = (ts c1) x (tq c1). Head pairs run CONCURRENTLY on
            the PE (K=64 each, auto row-group tiling from base_partition 0/64).
            Causal mask applied post-exp by zeroing ts>tq on GpSimd."""
            qkT = st.pop(("qk", b))
            attnT = sb.tile([128, 6, 384], BF16, tag="attnT", bufs=6)
            for m in range(3):
                pair = []
                for sub in range(2):
                    h = 2 * m + sub
                    po = sub * 64
                    qh = qkT[po : po + 64, m, :]
                    kh = qkT[po : po + 64, 3 + m, :]
                    S = ps_sc.tile([128, 384], F32, tag="sc")
                    pair.append((h, S, qh, kh))
                for h, S, qh, kh in pair:
                    nc.tensor.matmul(S[:, 0:256], lhsT=kh[:, 0:128], rhs=qh,
                                     start=True, stop=False)
                for h, S, qh, kh in pair:
                    nc.tensor.matmul(S[:, 256:384], lhsT=kh[:, 128:256],
                                     rhs=qh[:, 128:256], start=False, stop=True)
                for h, S, qh, kh in pair:
                    nc.scalar.activation(out=attnT[:, h, :], in_=S, func=AF.Exp)
                    # zero masked (ts > tq) entries of blocks 0:128 and 256:384
                    for blk in (attnT[:, h, 0:128], attnT[:, h, 256:384]):
                        nc.gpsimd.affine_select(
                            out=blk, in_=blk, compare_op=ALU.is_ge, fill=0.0,
                            base=0, pattern=[[1, 128]], channel_multiplier=-1,
                        )
            st[("at", b)] = attnT

        def s_rs(b):
            """rowsum reciprocals -> DRAM bounce broadcast to head-pair rows."""
            attnT = st[("at", b)]
            rs_ps = ps_sc.tile([6, 256], F32, tag="sc")
            for h in range(H):
                nc.tensor.matmul(rs_ps, lhsT=sel6[:, h, :], rhs=attnT[:, h, 0:256],
                                 start=(h == 0), stop=False, skip_group_check=True)
                nc.tensor.matmul(rs_ps[:, 128:256], lhsT=sel6[:, h, :],
                                 rhs=attnT[:, h, 256:384],
                                 start=False, stop=(h == H - 1), skip_group_check=True)
            # 1/d via exp(-ln d) on ACT: keeps the evac off the DVE queue and
            # both funcs stay in the pinned table set
            lnd = stats.tile([6, 256], F32, tag="lnd")
            nc.scalar.activation(out=lnd, in_=rs_ps, func=AF.Ln)
            rsi = stats.tile([6, 256], F32, tag="rsi")
            nc.scalar.activation(out=rsi, in_=lnd, func=AF.Exp, scale=-1.0)
            nc.sync.dma_start(out=rs_scr[b], in_=rsi)

        def s_bc(b):
            bcs = []
            for m in range(3):
                src_ap = rs_scr[b, 2 * m : 2 * m + 2, :]
                src = bass.AP(tensor=src_ap.tensor, offset=src_ap.offset,
                              ap=[list(src_ap.ap[0]), [0, 64], list(src_ap.ap[1])])
                bc_m = sb.tile([128, 256], F32, tag=f"rsbc{m}", bufs=3)
                nc.sync.dma_start(out=bc_m, in_=src)
                bcs.append(bc_m)
            st[("bc", b)] = bcs

        def s_av(b):
            attnT = st.pop(("at", b))
            v_sb = st.pop(("v", b))
            bcs = st.pop(("bc", b))
            oTs = sb.tile([128, 3, T], BF16, tag="oTs", bufs=4)
            oTr = sb.tile([128, 3, T], F32, tag="oTr", bufs=2)
            for m in range(3):
                oT_ps = ps_ot.tile([128, T], F32, tag="ot")
                for sub in range(2):
                    h = 2 * m + sub
                    po = sub * 64
                    tp = (0, po)
                    nc.tensor.matmul(oT_ps[po : po + 64, :],
                                     lhsT=v_sb[0][:, HD * h : HD * (h + 1)],
                                     rhs=attnT[:, h, 0:256],
                                     start=True, stop=False, tile_position=tp)
                    nc.tensor.matmul(oT_ps[po : po + 64, 128:256],
                                     lhsT=v_sb[1][:, HD * h : HD * (h + 1)],
                                     rhs=attnT[:, h, 256:384],
                                     start=False, stop=True, tile_position=tp)
                # evac on ACT frees the ot bank without waiting on the DVE queue
                nc.scalar.copy(out=oTr[:, m, :], in_=oT_ps)
            for m in range(3):
                nc.vector.tensor_mul(out=oTs[:, m, :], in0=oTr[:, m, :], in1=bcs[m])
            st[("o", b)] = oTs

        def s_pr(b):
            oTs = st.pop(("o", b))
            x_t = st.pop(("x", b))
            npj = 4 if "bp" in bias_sb else 3
            x2_t = []
            for tt in range(2):
                ps = ps_mm.tile([128, D], F32, tag="mm")
                for c in range(3):
                    nc.tensor.matmul(ps, lhsT=oTs[:, c, 128 * tt : 128 * (tt + 1)],
                                     rhs=wp_sb[:, c, :], start=(c == 0), stop=(c == npj - 1))
                if "bp" in bias_sb:
                    nc.tensor.matmul(ps, lhsT=ones[:, :128], rhs=bias_sb["bp"],
                                     start=False, stop=True)
                x2 = sbx.tile([128, D], F32, tag=f"x2_{tt}", bufs=6)
                nc.vector.tensor_add(out=x2, in0=x_t[tt], in1=ps)
                x2_t.append(x2)
            st[("h2", b)] = ln_pre(x2_t, "b")
            st[("x2", b)] = x2_t

        def s_tr2(b):
            st[("gT", b)] = ln_tr(st.pop(("h2", b)), "g")

        def s_f1(b):
            h2T = st.pop(("gT", b))
            fT = sb.tile([128, 12, T], F8E4, tag="fT", bufs=4)
            nf = 4 if "b1" in bias_sb else 3
            for mp in range(6):
                ps = ps_mm.tile([128, 512], F32, tag="mm")
                for sub in range(2):
                    m = 2 * mp + sub
                    reg = ps[:, 256 * sub : 256 * (sub + 1)]
                    for c in range(3):
                        nc.tensor.matmul(reg, lhsT=w1_sb[:, c, 128 * m : 128 * (m + 1)],
                                         rhs=h2T[:, c, :], start=(c == 0), stop=(c == nf - 1))
                    if "b1" in bias_sb:
                        nc.tensor.matmul(reg, lhsT=bias_sb["b1"][:, 128 * m : 128 * (m + 1)],
                                         rhs=ones[:, :T], start=False, stop=True)
                if mp % 2 == 0:
                    nc.scalar.activation(out=fT[:, 2 * mp : 2 * mp + 2, :], in_=ps,
                                         func=AF.Relu)
                else:
                    nc.vector.tensor_scalar_max(out=fT[:, 2 * mp : 2 * mp + 2, :],
                                                in0=ps, scalar1=0.0)
            st[("f", b)] = fT

        def s_f2(b):
            fT = st.pop(("f", b))
            x2_t = st.pop(("x2", b))
            for tt in range(2):
                ps = ps_mm.tile([128, D], F32, tag="mm")
                for kp in range(6):
                    kc = 2 * kp
                    nc.tensor.matmul(
                        ps,
                        lhsT=fT[:, kc : kc + 2, 128 * tt : 128 * (tt + 1)],
                        rhs=w2_sb[:, kc : kc + 2, :],
                        start=(kp == 0), stop=(kp == 5), perf_mode=DR,
                    )
                if "b2" in bias_sb:
                    # b2 pre-scaled by 256 host-side; K=1 bf16 matmul into group
                    nc.tensor.matmul(ps, lhsT=ones[:1, :128], rhs=bias_sb["b2"],
                                     start=False, stop=True, skip_group_check=True)
                ffs = sb.tile([128, D], F32, tag=f"ffs{tt}", bufs=2)
                nc.scalar.activation(out=ffs, in_=ps, func=AF.Identity,
                                     scale=1.0 / (FSCALE * FSCALE))
                fo = sb.tile([128, D], F32, tag=f"fo{tt}", bufs=2)
                nc.gpsimd.tensor_add(out=fo, in0=x2_t[tt], in1=ffs)
                nc.sync.dma_start(out=out_d[b, 128 * tt : 128 * (tt + 1), :], in_=fo)

        STAGES = [s_x, s_ln1, s_tr1, s_qkv, s_sc, s_rs, s_bc, s_av, s_pr, s_tr2, s_f1, s_f2]

        def emit_all():
            for s in range(SB + N_STAGES - 1):
                for k, fn in enumerate(STAGES):
                    b = s - k
                    if 0 <= b < SB:
                        fn(b)

        if reps == 1:
            emit_all()
        else:
            with tc.For_i(0, reps) as _:
                emit_all()


def prep_weights(Wq, Wk, Wv, Wp, bp, W1, b1, W2, b2, g1, be1, g2, be2):
    """Host-side weight folding. Returns dict of device arrays + bias flags."""
    import ml_dtypes
    bf = ml_dtypes.bfloat16
    f8 = ml_dtypes.float8_e4m3
    Wq = np.asarray(Wq, np.float32)
    Wk = np.asarray(Wk, np.float32)
    Wv = np.asarray(Wv, np.float32)
    Wp = np.asarray(Wp, np.float32)
    W1 = np.asarray(W1, np.float32)
    W2 = np.asarray(W2, np.float32)
    g1 = np.asarray(g1, np.float32); be1 = np.asarray(be1, np.float32)
    g2 = np.asarray(g2, np.float32); be2 = np.asarray(be2, np.float32)
    bp = np.asarray(bp, np.float32); b1 = np.asarray(b1, np.float32)
    b2 = np.asarray(b2, np.float32)

    # [H, D, HD] -> [D, H*HD]
    Wq2 = Wq.transpose(1, 0, 2).reshape(D, D)
    Wk2 = Wk.transpose(1, 0, 2).reshape(D, D)
    Wv2 = Wv.transpose(1, 0, 2).reshape(D, D)
    Wqkv = np.concatenate([Wq2, Wk2, Wv2], axis=1)          # [D, 3D]
    bqkv = be1 @ Wqkv                                       # bias from LN1 beta
    Wqkv = g1[:, None] * Wqkv                               # fold LN1 gamma
    scale = 1.0 / np.sqrt(np.float32(D))
    Wqkv[:, :D] *= scale                                    # fold score scale into q
    bqkv = bqkv.copy()
    bqkv[:D] *= scale

    W1e = g2[:, None] * W1 * FSCALE                         # fold LN2 gamma + fp8 scale
    b1e = (b1 + be2 @ W1) * FSCALE                          # fold LN2 beta

    W2s = np.clip(W2 * FSCALE, -240.0, 240.0)               # fp8 range

    out = {
        "wqkv": np.ascontiguousarray(Wqkv.reshape(3, 128, 3 * D)).astype(bf),
        "wp": np.ascontiguousarray(Wp.reshape(3, 128, D)).astype(bf),
        "w1": np.ascontiguousarray(W1e.reshape(3, 128, DF)).astype(bf),
        "w2": np.ascontiguousarray(W2s.reshape(12, 128, D)).astype(f8),
    }
    flags = {}
    b2s = b2 * (FSCALE * FSCALE)  # survives the 1/256 evac rescale
    for name, arr in (("bqkv", bqkv), ("bp", bp), ("b1", b1e), ("b2", b2s)):
        if np.any(arr != 0):
            out[name] = arr.reshape(1, -1).astype(bf)
            flags[f"use_{name}"] = True
        else:
            flags[f"use_{name}"] = False
    return out, flags


_CACHE = {}


def get_program(flags, reps=1):
    key = (reps, tuple(sorted(flags.items())))
    if key not in _CACHE:
        _CACHE[key] = build_program(reps=reps, **flags)
    return _CACHE[key]


def make_in_maps(x, w):
    in_maps = []
    for c in range(N_CORES):
        m = {"x": np.ascontiguousarray(np.asarray(x, np.float32)[c * SB : (c + 1) * SB])}
        m.update(w)
        in_maps.append(m)
    return in_maps


def kernel(x, Wq, Wk, Wv, Wp, bp, W1, b1, W2, b2, g1, be1, g2, be2):
    from concourse.bass_utils import run_bass_kernel_spmd

    w, flags = prep_weights(Wq, Wk, Wv, Wp, bp, W1, b1, W2, b2, g1, be1, g2, be2)
    nc = get_program(flags, reps=1)
    in_maps = make_in_maps(x, w)
    res = run_bass_kernel_spmd(nc, in_maps, list(range(N_CORES)))
    return np.concatenate([res.results[c]["out"] for c in range(N_CORES)], axis=0)
